# revision 6
# baseline (speedup 1.0000x reference)
"""Kalman filter estimator (nn_KalmanFilterEstimator) as a Bass/Tile kernel on 8 TRN2 cores.

Reformulation: the scan is linear in the data once the (data-independent) Riccati
gain sequence is known. With x0 = 0:

    x_{t+1} = x_t @ Aeff_t + c_t,
    c_t     = u_t @ (B_W G_t) + d_t @ (E_W G_t) + ym_t @ Lc_t^T,
    G_t     = I - C_W @ Lc_t^T,   Aeff_t = A_W @ G_t,

so x_T = sum_t c_t @ (Aeff_{t+1} ... Aeff_{T-1}).  The gain converges to Lbar in
~46 steps (rho(Abar) ~ 0.73), so Aeff_t == Abar (f32-exactly) beyond the first few
steps, and the suffix product is Abar^(T-1-t).  Contributions older than ~150 steps
vanish below f32 epsilon (rho^256 ~ 4e-35), which also makes the tiny pre-converged
prefix irrelevant.  The device computes, per core m over its 256-step time slice:

    partial_m = sum_b [ sum_a Z_t(a,b) @ W_a ] @ MB_{m,b}
    W_a  = [B_W G; E_W G; Lbar^T] @ Abar^a          (stacked, [128 x 128])
    MB_{m,b} = Abar^(16 b + 256 (7 - m))
    Z_t  = [u_t ; d_t ; ym_t]    (stacked features, transposed to [128 feat x 128 B])

Everything on device is a [K=128] x [N] matmul accumulated in PSUM; the time axis is
sharded across the 8 cores and the 8 [NX x B] partials are summed on the host.
"""

import numpy as np

NX, NY, NU, ND = 128, 64, 32, 32
T, B = 2048, 128
HEAT_C = 0.997 * 4185.5 * (1.0 / 3600.0)
N_CORES = 8
TC = T // N_CORES          # 256 timesteps per core
NA = 16                    # inner radix  (Abar^a,  a in [0,16))
NB = 16                    # outer radix  (Abar^16b)
NSG = 4                    # supergroups of 4 blocks; 16 matmuls of N=512 each
_cache = {}


def _build_weights(A_W, B_W, E_W, C_W, Q, R, P0, L0):
    """Riccati recursion in float64 -> folded steady-state weights (f32)."""
    A = A_W.astype(np.float64); C = C_W.astype(np.float64)
    Qf = Q.astype(np.float64); Rf = R.astype(np.float64)
    eye = np.eye(NX)
    P = P0.astype(np.float64); L = L0.astype(np.float64)
    prev = None
    for t in range(300):
        P_pred = A @ P @ A.T + Qf
        S = Rf + C.T @ P_pred @ C
        L = P_pred @ C @ np.linalg.inv(S)
        P = eye - L @ (C.T @ P_pred)
        if prev is not None and np.linalg.norm(L - prev) <= 1e-13 * np.linalg.norm(L):
            break
        prev = L.copy()
    G = eye - C @ L.T
    Abar = A @ G
    rho = np.abs(np.linalg.eigvals(Abar)).max()
    assert rho < 0.98, f"closed-loop not contracting enough (rho={rho})"
    SW = np.concatenate([B_W.astype(np.float64) @ G,
                         E_W.astype(np.float64) @ G,
                         L.T], axis=0)                     # [128, NX]
    Apow = np.eye(NX)
    W_cols = []
    for a in range(NA):
        W_cols.append((SW @ Apow).astype(np.float32))
        Apow = Apow @ Abar
    WA = np.concatenate(W_cols, axis=1)                    # [128, NA*128]
    A16 = Apow                                             # Abar^16
    A16p = np.eye(NX)
    allp = []
    for k in range(NB * N_CORES):
        allp.append(A16p.astype(np.float32))
        A16p = A16p @ A16
    MB = np.zeros((N_CORES, NX, NB * NX), np.float32)
    for m in range(N_CORES):
        for b in range(NB):
            MB[m][:, b * NX:(b + 1) * NX] = allp[b + NB * (N_CORES - 1 - m)]
    return WA, MB


def _build_bass():
    import concourse.bacc as bacc
    import concourse.mybir as mybir
    from concourse.tile import TileContext

    f32 = mybir.dt.float32
    nc = bacc.Bacc(None, target_bir_lowering=False)
    # Per-core SBUF-image of the data slice: [128 feat, TC*B] with column order
    # (supergroup s, within-block pos q, block-in-supergroup kl, batch b).
    zc = nc.dram_tensor("zc", [128, TC * B], f32, kind="ExternalInput")
    wa = nc.dram_tensor("wa", [128, NA * 128], f32, kind="ExternalInput")
    mb = nc.dram_tensor("mb", [128, NB * 128], f32, kind="ExternalInput")
    out = nc.dram_tensor("out", [128, B], f32, kind="ExternalOutput")

    COLS_SG = (TC // NSG) * B            # 8192 columns per supergroup
    with TileContext(nc) as tc:
        with (
            tc.tile_pool(name="wpool", bufs=1) as wpool,
            tc.tile_pool(name="zpool", bufs=2) as zpool,
            tc.tile_pool(name="gsb", bufs=2) as gsb_pool,
            tc.tile_pool(name="acc", bufs=1) as acc_pool,
            tc.tile_pool(name="gpsum", bufs=2, space="PSUM") as gpsum_pool,
            tc.tile_pool(name="ppsum", bufs=2, space="PSUM") as ppsum_pool,
        ):
            w_tile = wpool.tile([128, NA * 128], f32, tag="wa")
            nc.gpsimd.dma_start(out=w_tile[:, :], in_=wa[:, :])
            mb_tile = wpool.tile([128, NB * 128], f32, tag="mb")
            nc.gpsimd.dma_start(out=mb_tile[:, :], in_=mb[:, :])

            part_sb = []
            for s in range(NSG):
                zbuf = zpool.tile([128, COLS_SG], f32)
                nc.gpsimd.dma_start(
                    out=zbuf[:, :],
                    in_=zc[:, s * COLS_SG:(s + 1) * COLS_SG],
                )
                g4 = gpsum_pool.tile([128, 4 * B], f32)
                for q in range(16):
                    # ascending-t position q inside each block uses W_{15-q}
                    a = 15 - q
                    nc.tensor.matmul(
                        g4,
                        w_tile[:, a * 128:(a + 1) * 128],
                        zbuf[:, q * 4 * B:(q + 1) * 4 * B],
                        start=(q == 0), stop=(q == 15),
                    )
                g4_sb = gsb_pool.tile([128, 4 * B], f32)
                nc.vector.tensor_copy(out=g4_sb, in_=g4)
                pps = ppsum_pool.tile([128, B], f32)
                for kl in range(4):
                    blk = s * 4 + kl          # ascending-t block index
                    b_idx = NB - 1 - blk      # group b = 15 - block
                    nc.tensor.matmul(
                        pps,
                        mb_tile[:, b_idx * 128:(b_idx + 1) * 128],
                        g4_sb[:, kl * B:(kl + 1) * B],
                        start=(kl == 0), stop=(kl == 3),
                    )
                psb = acc_pool.tile([128, B], f32, tag=f"part{s}")
                nc.vector.tensor_copy(out=psb, in_=pps)
                part_sb.append(psb)

            s01 = acc_pool.tile([128, B], f32, tag="s01")
            s23 = acc_pool.tile([128, B], f32, tag="s23")
            nc.vector.tensor_add(out=s01, in0=part_sb[0], in1=part_sb[1])
            nc.vector.tensor_add(out=s23, in0=part_sb[2], in1=part_sb[3])
            tot = acc_pool.tile([128, B], f32, tag="tot")
            nc.vector.tensor_add(out=tot, in0=s01, in1=s23)
            nc.sync.dma_start(out=out[:, :], in_=tot[:, :])
    nc.finalize()
    return nc


def _pack_z(Ym, M_flow, DT, D):
    """Build per-core SBUF-image arrays [128, TC*B] (f32, contiguous).

    Column order: s-major, then q (pos in block, ascending t), then kl (block in
    supergroup), then batch.  Global t of (m, s, q, kl) = m*TC + (s*4+kl)*16 + q.
    """
    u = (np.float32(HEAT_C) * M_flow * DT).astype(np.float32)
    Z = np.concatenate([u, D, Ym], axis=2)          # [T, B, 128]
    ZT = Z.transpose(0, 2, 1)                       # [T, 128, B] (view)
    # [N_CORES, NSG, blocks(kl)=4, q=16, 128, B] from t = ((m*4+s)*4+kl)*16+q
    Z6 = ZT.reshape(N_CORES, NSG, 4, 16, 128, B)
    # -> (m, 128, s, q, kl, B)
    Zp = np.ascontiguousarray(Z6.transpose(0, 4, 1, 3, 2, 5))
    return Zp.reshape(N_CORES, 128, TC * B)


def kernel(Ym, M_flow, DT, D, A_W, B_W, E_W, C_W, Q, R, P0, L0, x0):
    from concourse.bass_utils import run_bass_kernel_spmd

    if "nc" not in _cache:
        _cache["nc"] = _build_bass()
    nc = _cache["nc"]

    WA, MB = _build_weights(A_W, B_W, E_W, C_W, Q, R, P0, L0)
    Zp = _pack_z(Ym, M_flow, DT, D)
    in_maps = [
        {"zc": Zp[m], "wa": WA, "mb": MB[m]}
        for m in range(N_CORES)
    ]
    res = run_bass_kernel_spmd(nc, in_maps, core_ids=list(range(N_CORES)))
    xT = np.zeros((NX, B), np.float32)
    for m in range(N_CORES):
        xT += res.results[m]["out"]
    # account for x0 in case it is nonzero: x0 @ Aeff_0 ... == x0 @ ~0 for our
    # contraction rates; reference x0 is zeros so nothing to add.
    return np.ascontiguousarray(xT.T)


# revision 7
# speedup vs baseline: 3.0359x; 3.0359x over previous
"""Kalman filter estimator (nn_KalmanFilterEstimator) as a Bass/Tile kernel on 8 TRN2 cores.

Reformulation: the scan is linear in the data once the (data-independent) Riccati
gain sequence is known. With x0 = 0:

    x_{t+1} = x_t @ Aeff_t + c_t,
    c_t     = u_t @ (B_W G_t) + d_t @ (E_W G_t) + ym_t @ Lc_t^T,
    G_t     = I - C_W @ Lc_t^T,   Aeff_t = A_W @ G_t,

so x_T = sum_t c_t @ (Aeff_{t+1} ... Aeff_{T-1}).  The gain converges to Lbar in
~46 steps (rho(Abar) ~ 0.73, checked at runtime), so Aeff_t == Abar beyond the
first few steps and the suffix product is Abar^(T-1-t).  Contributions decay as
rho^age: anything older than ~330 steps underflows to exactly 0 in float32 (the
reference output provably cannot depend on it).  We therefore compute

    x_T = sum_{t >= T-WIN} c_t @ Abar^(T-1-t),        WIN = 256
        (truncation error ~ rho^WIN ~ 4e-35  <<  f32 epsilon)

time-sharded over 8 cores (32 steps each).  Per core m, with 16-step blocks:

    partial_m = sum_{kl<2} [ sum_{q<16} Z_{t(kl,q)} @ W_{15-q} ] @ MB_{m,kl}
    W_a      = [B_W G; E_W G; Lbar^T] @ Abar^a            ([128 x 128], stacked)
    MB_{m,kl} = Abar^(16 (1-kl) + 32 (7-m))
    Z_t      = [u_t ; d_t ; ym_t] transposed to [128 feat x 128 batch]

All device work is K=128 matmuls accumulated in PSUM (inner stage fuses the two
blocks into N=256 moving operands); the 8 [NX x B] partials are summed on host.
Weight-only precompute (Riccati, matrix powers) runs on host in float64.
"""

import numpy as np

NX, NY, NU, ND = 128, 64, 32, 32
T, B = 2048, 128
HEAT_C = 0.997 * 4185.5 * (1.0 / 3600.0)
N_CORES = 8
WIN = 256                  # time window that fully determines x_T at f32
TCW = WIN // N_CORES       # 32 timesteps per core
NBW = TCW // 16            # 2 blocks of 16 steps per core
NA = 16                    # inner radix (Abar^a, a in [0,16))
_cache = {}


def _build_weights(A_W, B_W, E_W, C_W, Q, R, P0, L0):
    """Riccati recursion in float64 -> folded steady-state weights (f32)."""
    A = A_W.astype(np.float64); C = C_W.astype(np.float64)
    Qf = Q.astype(np.float64); Rf = R.astype(np.float64)
    eye = np.eye(NX)
    P = P0.astype(np.float64); L = L0.astype(np.float64)
    prev = None
    for t in range(300):
        P_pred = A @ P @ A.T + Qf
        S = Rf + C.T @ P_pred @ C
        L = P_pred @ C @ np.linalg.inv(S)
        P = eye - L @ (C.T @ P_pred)
        if prev is not None and np.linalg.norm(L - prev) <= 1e-13 * np.linalg.norm(L):
            break
        prev = L.copy()
    G = eye - C @ L.T
    Abar = A @ G
    rho = np.abs(np.linalg.eigvals(Abar)).max()
    # window must annihilate truncated history far below f32 resolution
    assert rho ** WIN < 1e-20, f"decay too slow for WIN={WIN} (rho={rho})"
    SW = np.concatenate([B_W.astype(np.float64) @ G,
                         E_W.astype(np.float64) @ G,
                         L.T], axis=0)                     # [128, NX]
    Apow = np.eye(NX)
    W_cols = []
    for a in range(NA):
        W_cols.append((SW @ Apow).astype(np.float32))
        Apow = Apow @ Abar
    WA = np.concatenate(W_cols, axis=1)                    # [128, NA*128]
    MB = np.zeros((N_CORES, NX, NBW * NX), np.float32)
    for m in range(N_CORES):
        for kl in range(NBW):   # block ascending in t inside the core slice
            e = 16 * (NBW - 1 - kl) + TCW * (N_CORES - 1 - m)
            MB[m][:, kl * NX:(kl + 1) * NX] = np.linalg.matrix_power(
                Abar, e).astype(np.float32)
    return WA, MB


def _build_bass():
    import concourse.bacc as bacc
    import concourse.mybir as mybir
    from concourse.tile import TileContext

    f32 = mybir.dt.float32
    nc = bacc.Bacc(None, target_bir_lowering=False)
    # Per-core SBUF-image of the data slice: [128 feat, TCW*B] with column order
    # (q = pos in block ascending t, kl = block, batch).
    zc = nc.dram_tensor("zc", [128, TCW * B], f32, kind="ExternalInput")
    wa = nc.dram_tensor("wa", [128, NA * 128], f32, kind="ExternalInput")
    mb = nc.dram_tensor("mb", [128, NBW * 128], f32, kind="ExternalInput")
    out = nc.dram_tensor("out", [128, B], f32, kind="ExternalOutput")

    NW = NBW * B                        # moving-operand width of inner matmuls
    with TileContext(nc) as tc:
        with (
            tc.tile_pool(name="wpool", bufs=1) as wpool,
            tc.tile_pool(name="zpool", bufs=1) as zpool,
            tc.tile_pool(name="gsb", bufs=1) as gsb_pool,
            tc.tile_pool(name="gpsum", bufs=1, space="PSUM") as gpsum_pool,
            tc.tile_pool(name="ppsum", bufs=1, space="PSUM") as ppsum_pool,
        ):
            w_tile = wpool.tile([128, NA * 128], f32, tag="wa")
            nc.sync.dma_start(out=w_tile[:, :], in_=wa[:, :])
            mb_tile = wpool.tile([128, NBW * 128], f32, tag="mb")
            nc.sync.dma_start(out=mb_tile[:, :], in_=mb[:, :])
            zbuf = zpool.tile([128, TCW * B], f32)
            HALF = TCW * B // 2
            nc.sync.dma_start(out=zbuf[:, :HALF], in_=zc[:, :HALF])
            nc.sync.dma_start(out=zbuf[:, HALF:], in_=zc[:, HALF:])

            g2 = gpsum_pool.tile([128, NW], f32)
            for q in range(16):
                # ascending-t position q inside each block uses W_{15-q}
                a = 15 - q
                nc.tensor.matmul(
                    g2,
                    w_tile[:, a * 128:(a + 1) * 128],
                    zbuf[:, q * NW:(q + 1) * NW],
                    start=(q == 0), stop=(q == 15),
                )
            g2_sb = gsb_pool.tile([128, NW], f32)
            nc.vector.tensor_copy(out=g2_sb, in_=g2)
            pps = ppsum_pool.tile([128, B], f32)
            for kl in range(NBW):
                nc.tensor.matmul(
                    pps,
                    mb_tile[:, kl * 128:(kl + 1) * 128],
                    g2_sb[:, kl * B:(kl + 1) * B],
                    start=(kl == 0), stop=(kl == NBW - 1),
                )
            tot = gsb_pool.tile([128, B], f32, tag="tot")
            nc.vector.tensor_copy(out=tot, in_=pps)
            nc.sync.dma_start(out=out[:, :], in_=tot[:, :])
    nc.finalize()
    return nc


def _pack_z(Ym, M_flow, DT, D):
    """Per-core SBUF-image arrays [128, TCW*B] (f32, contiguous) for the last
    WIN timesteps.  Column order (q, kl, b); t = (T-WIN) + m*TCW + kl*16 + q."""
    lo = T - WIN
    u = (np.float32(HEAT_C) * M_flow[lo:] * DT[lo:]).astype(np.float32)
    Z = np.concatenate([u, D[lo:], Ym[lo:]], axis=2)   # [WIN, B, 128]
    ZT = Z.transpose(0, 2, 1)                          # [WIN, 128, B] (view)
    Z5 = ZT.reshape(N_CORES, NBW, 16, 128, B)          # (m, kl, q, feat, b)
    Zp = np.ascontiguousarray(Z5.transpose(0, 3, 2, 1, 4))   # (m, feat, q, kl, b)
    return Zp.reshape(N_CORES, 128, TCW * B)


def kernel(Ym, M_flow, DT, D, A_W, B_W, E_W, C_W, Q, R, P0, L0, x0):
    from concourse.bass_utils import run_bass_kernel_spmd

    if "nc" not in _cache:
        _cache["nc"] = _build_bass()
    nc = _cache["nc"]

    WA, MB = _build_weights(A_W, B_W, E_W, C_W, Q, R, P0, L0)
    Zp = _pack_z(Ym, M_flow, DT, D)
    in_maps = [{"zc": Zp[m], "wa": WA, "mb": MB[m]} for m in range(N_CORES)]
    res = run_bass_kernel_spmd(nc, in_maps, core_ids=list(range(N_CORES)))
    xT = np.zeros((NX, B), np.float32)
    for m in range(N_CORES):
        xT += res.results[m]["out"]
    # x0 is zeros in this model; if it were not, its influence decays by
    # Abar^T ~ 0 anyway at f32.
    return np.ascontiguousarray(xT.T)


# revision 8
# speedup vs baseline: 3.0749x; 1.0129x over previous
"""Kalman filter estimator (nn_KalmanFilterEstimator) as a Bass/Tile kernel on 8 TRN2 cores.

Reformulation: the scan is linear in the data once the (data-independent) Riccati
gain sequence is known. With x0 = 0:

    x_{t+1} = x_t @ Aeff_t + c_t,
    c_t     = u_t @ (B_W G_t) + d_t @ (E_W G_t) + ym_t @ Lc_t^T,
    G_t     = I - C_W @ Lc_t^T,   Aeff_t = A_W @ G_t,

so x_T = sum_t c_t @ (Aeff_{t+1} ... Aeff_{T-1}).  The gain converges to Lbar in
~46 steps (rho(Abar) ~ 0.73, checked at runtime), so Aeff_t == Abar beyond the
first few steps and the suffix product is Abar^(T-1-t).  Contributions decay as
rho^age: anything older than ~330 steps underflows to exactly 0 in float32 (the
reference output provably cannot depend on it).  We therefore compute

    x_T = sum_{t >= T-WIN} c_t @ Abar^(T-1-t),        WIN = 256
        (truncation error ~ rho^WIN ~ 4e-35  <<  f32 epsilon)

time-sharded over 8 cores (32 steps each).  Per core m, with 16-step blocks:

    partial_m = sum_{kl<2} [ sum_{q<16} Z_{t(kl,q)} @ W_{15-q} ] @ MB_{m,kl}
    W_a      = [B_W G; E_W G; Lbar^T] @ Abar^a            ([128 x 128], stacked)
    MB_{m,kl} = Abar^(16 (1-kl) + 32 (7-m))
    Z_t      = [u_t ; d_t ; ym_t] transposed to [128 feat x 128 batch]

All device work is K=128 matmuls accumulated in PSUM (inner stage fuses the two
blocks into N=256 moving operands); the 8 [NX x B] partials are summed on host.
Weight-only precompute (Riccati, matrix powers) runs on host in float64.
"""

import numpy as np

NX, NY, NU, ND = 128, 64, 32, 32
T, B = 2048, 128
HEAT_C = 0.997 * 4185.5 * (1.0 / 3600.0)
N_CORES = 8
WIN = 256                  # time window that fully determines x_T at f32
TCW = WIN // N_CORES       # 32 timesteps per core
NBW = TCW // 16            # 2 blocks of 16 steps per core
NA = 16                    # inner radix (Abar^a, a in [0,16))
_cache = {}


def _build_weights(A_W, B_W, E_W, C_W, Q, R, P0, L0):
    """Riccati recursion in float64 -> folded steady-state weights (f32)."""
    A = A_W.astype(np.float64); C = C_W.astype(np.float64)
    Qf = Q.astype(np.float64); Rf = R.astype(np.float64)
    eye = np.eye(NX)
    P = P0.astype(np.float64); L = L0.astype(np.float64)
    prev = None
    for t in range(300):
        P_pred = A @ P @ A.T + Qf
        S = Rf + C.T @ P_pred @ C
        L = P_pred @ C @ np.linalg.inv(S)
        P = eye - L @ (C.T @ P_pred)
        if prev is not None and np.linalg.norm(L - prev) <= 1e-13 * np.linalg.norm(L):
            break
        prev = L.copy()
    G = eye - C @ L.T
    Abar = A @ G
    rho = np.abs(np.linalg.eigvals(Abar)).max()
    # window must annihilate truncated history far below f32 resolution
    assert rho ** WIN < 1e-20, f"decay too slow for WIN={WIN} (rho={rho})"
    SW = np.concatenate([B_W.astype(np.float64) @ G,
                         E_W.astype(np.float64) @ G,
                         L.T], axis=0)                     # [128, NX]
    Apow = np.eye(NX)
    W_cols = []
    for a in range(NA):
        W_cols.append((SW @ Apow).astype(np.float32))
        Apow = Apow @ Abar
    WA = np.concatenate(W_cols, axis=1)                    # [128, NA*128]
    MB = np.zeros((N_CORES, NX, NBW * NX), np.float32)
    for m in range(N_CORES):
        for kl in range(NBW):   # block ascending in t inside the core slice
            e = 16 * (NBW - 1 - kl) + TCW * (N_CORES - 1 - m)
            MB[m][:, kl * NX:(kl + 1) * NX] = np.linalg.matrix_power(
                Abar, e).astype(np.float32)
    return WA, MB


def _build_bass():
    import concourse.bacc as bacc
    import concourse.mybir as mybir
    from concourse.tile import TileContext

    f32 = mybir.dt.float32
    nc = bacc.Bacc(None, target_bir_lowering=False)
    # Per-core SBUF-image of the data slice: [128 feat, TCW*B] with column order
    # (q = pos in block ascending t, kl = block, batch).
    zc = nc.dram_tensor("zc", [128, TCW * B], f32, kind="ExternalInput")
    wa = nc.dram_tensor("wa", [128, NA * 128], f32, kind="ExternalInput")
    mb = nc.dram_tensor("mb", [128, NBW * 128], f32, kind="ExternalInput")
    out = nc.dram_tensor("out", [128, B], f32, kind="ExternalOutput")

    NW = NBW * B                        # moving-operand width of inner matmuls
    with TileContext(nc) as tc:
        with (
            tc.tile_pool(name="wpool", bufs=1) as wpool,
            tc.tile_pool(name="zpool", bufs=1) as zpool,
            tc.tile_pool(name="gsb", bufs=1) as gsb_pool,
            tc.tile_pool(name="gpsum", bufs=1, space="PSUM") as gpsum_pool,
            tc.tile_pool(name="ppsum", bufs=1, space="PSUM") as ppsum_pool,
        ):
            # spread the loads across independent DMA rings so they overlap:
            # sync (HWDGE) gets the weights, scalar (HWDGE) the first data
            # half (needed by matmul q=0), sync the second, gpsimd the tiny mb
            w_tile = wpool.tile([128, NA * 128], f32, tag="wa")
            nc.sync.dma_start(out=w_tile[:, :], in_=wa[:, :])
            mb_tile = wpool.tile([128, NBW * 128], f32, tag="mb")
            nc.gpsimd.dma_start(out=mb_tile[:, :], in_=mb[:, :])
            zbuf = zpool.tile([128, TCW * B], f32)
            HALF = TCW * B // 2
            nc.scalar.dma_start(out=zbuf[:, :HALF], in_=zc[:, :HALF])
            nc.sync.dma_start(out=zbuf[:, HALF:], in_=zc[:, HALF:])

            g2 = gpsum_pool.tile([128, NW], f32)
            for q in range(16):
                # ascending-t position q inside each block uses W_{15-q}
                a = 15 - q
                nc.tensor.matmul(
                    g2,
                    w_tile[:, a * 128:(a + 1) * 128],
                    zbuf[:, q * NW:(q + 1) * NW],
                    start=(q == 0), stop=(q == 15),
                )
            g2_sb = gsb_pool.tile([128, NW], f32)
            nc.vector.tensor_copy(out=g2_sb, in_=g2)
            pps = ppsum_pool.tile([128, B], f32)
            for kl in range(NBW):
                nc.tensor.matmul(
                    pps,
                    mb_tile[:, kl * 128:(kl + 1) * 128],
                    g2_sb[:, kl * B:(kl + 1) * B],
                    start=(kl == 0), stop=(kl == NBW - 1),
                )
            tot = gsb_pool.tile([128, B], f32, tag="tot")
            nc.vector.tensor_copy(out=tot, in_=pps)
            nc.sync.dma_start(out=out[:, :], in_=tot[:, :])
    nc.finalize()
    return nc


def _pack_z(Ym, M_flow, DT, D):
    """Per-core SBUF-image arrays [128, TCW*B] (f32, contiguous) for the last
    WIN timesteps.  Column order (q, kl, b); t = (T-WIN) + m*TCW + kl*16 + q."""
    lo = T - WIN
    u = (np.float32(HEAT_C) * M_flow[lo:] * DT[lo:]).astype(np.float32)
    Z = np.concatenate([u, D[lo:], Ym[lo:]], axis=2)   # [WIN, B, 128]
    ZT = Z.transpose(0, 2, 1)                          # [WIN, 128, B] (view)
    Z5 = ZT.reshape(N_CORES, NBW, 16, 128, B)          # (m, kl, q, feat, b)
    Zp = np.ascontiguousarray(Z5.transpose(0, 3, 2, 1, 4))   # (m, feat, q, kl, b)
    return Zp.reshape(N_CORES, 128, TCW * B)


def kernel(Ym, M_flow, DT, D, A_W, B_W, E_W, C_W, Q, R, P0, L0, x0):
    from concourse.bass_utils import run_bass_kernel_spmd

    if "nc" not in _cache:
        _cache["nc"] = _build_bass()
    nc = _cache["nc"]

    WA, MB = _build_weights(A_W, B_W, E_W, C_W, Q, R, P0, L0)
    Zp = _pack_z(Ym, M_flow, DT, D)
    in_maps = [{"zc": Zp[m], "wa": WA, "mb": MB[m]} for m in range(N_CORES)]
    res = run_bass_kernel_spmd(nc, in_maps, core_ids=list(range(N_CORES)))
    xT = np.zeros((NX, B), np.float32)
    for m in range(N_CORES):
        xT += res.results[m]["out"]
    # x0 is zeros in this model; if it were not, its influence decays by
    # Abar^T ~ 0 anyway at f32.
    return np.ascontiguousarray(xT.T)
